# revision 36
# baseline (speedup 1.0000x reference)
"""BottleneckAttn TRN2 kernel.

Reference computation (per batch b, head n, fp32):
    qkv = w_qkv @ x_b                      # (1536, 1024), 1x1 conv
    q, k, v per head: (1024, 128) with hw = h*32 + w
    logits[q,k] = SCALE * (q . k) + qw[q, 31 + w2(k) - w(q)] + qh[q, 31 + h2(k) - h(q)]
        where qw[q,r] = q . width_rel[r], qh[q,r] = q . height_rel[r]
    out = softmax(logits) @ v              # (1024, 128)
    output[b] flat index = q*512 + n*128 + d  -> reshape (512, 32, 32)

Device strategy (SPMD, 8 cores, 2 batches/core):
  - All matmuls computed in the TRANSPOSED softmax layout ST[k, q] so the
    attention probabilities come out of the exp directly in the layout the
    PV matmul needs as its moving operand (no P transposes).
  - ST = k @ qT via PE (operands swapped); the relative-position bias is
    folded in as a second accumulating matmul with a constant 0/1 selection
    matrix lhsT (rows select the shifted width/height tables per PSUM
    partition).
  - The per-query shifted tables (skew gather) are built by a DRAM round
    trip: qw/qh computed in [q, r] layout on PE, stored to a DRAM scratch,
    re-loaded with an affine skewed access pattern (contiguous 32-element
    runs), then rotated into [table_row, q] layout with DVE 32x32 stream
    transposes.
  - Softmax denominators via an all-ones stationary matmul with M=1 outputs
    COLUMN-TILED over the four 32-wide PE column groups (tile_position):
    4 concurrent denominator matmuls per wave, 4 waves per head, partials
    at PSUM partitions 0/32/64/96 summed on the host after the gather.
  - EVERY matmul runs bf16: measured fastest issue rate (~216ns per
    512-col matmul vs 227ns f32r), but ONLY when the PE instruction
    stream is dtype-homogeneous — mixing fp16/f32r modes costs ~100ns
    per switch. The sel stationary is zero-padded to K=128: the 64-row
    (row_grp) PE configuration measured ~2x the issue interval.
  - PV matmuls for k-tile kc are emitted after the ST+bias matmuls of
    kc+1 so the scalar-engine exp has a full k-tile of slack to hide in.
  - The bias chain (q proj + qwh + DRAM skew trip + DVE transposes) is
    pipelined THREE heads ahead of attention so the DVE transpose queue
    never blocks the PE's sel matmuls.
  - x is pre-swizzled to the device's f-order hw columns on the host, so
    the projection matmuls read plain contiguous slices.
  - Startup loads are spread over the three DMA rings (sync/scalar/gpsimd)
    interleaved in first-use order.
"""

import os
import sys

import numpy as np

for _p in ("/opt/trn_rl_repo", "/root/.axon_site/_ro/trn_rl_repo"):
    if os.path.isdir(_p) and _p not in sys.path:
        sys.path.append(_p)

import ml_dtypes

import concourse.bass as bass
import concourse.mybir as mybir
import concourse.tile as tile
from concourse import bacc
from concourse.bass_utils import run_bass_kernel_spmd

B, C, H, W = 16, 512, 32, 32
HW = H * W
NH, DH = 4, 128
SCALE = DH ** -0.5
N_CORES = 8
B_LOC = B // N_CORES

F32 = mybir.dt.float32
F32R = mybir.dt.float32r
BF16 = mybir.dt.bfloat16
EXP = mybir.ActivationFunctionType.Exp

_CACHE = {}

# f-order permutation: device hw column f = 256*s + 32*qt + j for q = 128*qt + 32*s + j
_QS = np.arange(1024)
_F_OF_Q = (256 * ((_QS % 128) // 32) + 32 * (_QS // 128) + (_QS % 32)).astype(np.int64)
_Q_OF_F = np.argsort(_F_OF_Q)

BIAS_BUFS = 5  # bias_vecT ring size (pipeline depth 3 + slack)


def _sel_const():
    """sel[j, kc*128 + p]: j<32 selects shifted width row w2(p), j in
    [32,64) selects shifted height row h2(p). Rows 64..127 are ZERO
    padding: a full-height K=128 stationary avoids the slow 64-row
    (row_grp) PE configuration measured at ~2x the issue interval."""
    sel = np.zeros((128, 8 * 128), np.float32)
    for kc in range(8):
        for p in range(128):
            f = 128 * kc + p
            sg, qt, j = f // 256, (f % 256) // 32, f % 32
            q = 128 * qt + 32 * sg + j
            sel[q % 32, kc * 128 + p] = 1.0
            sel[32 + q // 32, kc * 128 + p] = 1.0
    return sel  # [128, 1024]


def _emit(tc, nc, xd, wd, reld, seld, onesd, identd, outd, dend, scr_handles):
    from contextlib import ExitStack

    ctx = ExitStack()
    with ctx:
        const = ctx.enter_context(tc.tile_pool(name="const", bufs=1))
        xpool = ctx.enter_context(tc.tile_pool(name="x", bufs=B_LOC))
        qkvp = ctx.enter_context(tc.tile_pool(name="qkv", bufs=BIAS_BUFS))
        qwhp = ctx.enter_context(tc.tile_pool(name="qwh", bufs=4))
        wvp = ctx.enter_context(tc.tile_pool(name="wv", bufs=4))
        biasp = ctx.enter_context(tc.tile_pool(name="biasv", bufs=BIAS_BUFS))
        vnatp = ctx.enter_context(tc.tile_pool(name="vnat", bufs=2))
        ptp = ctx.enter_context(tc.tile_pool(name="pt", bufs=12))
        outp = ctx.enter_context(tc.tile_pool(name="outt", bufs=2))
        recp = ctx.enter_context(tc.tile_pool(name="recip", bufs=2))
        # PSUM budget (8 banks): st 3 + out 2 + den 1 + misc 2. The third
        # st buffer keeps the ST matmuls from pacing at the exp rate (with
        # two, ST(kc+2) must wait for exp(kc) to release its bank).
        ps_st = ctx.enter_context(tc.tile_pool(name="psst", bufs=3, space="PSUM"))
        ps_out = ctx.enter_context(tc.tile_pool(name="psout", bufs=2, space="PSUM"))
        ps_den = ctx.enter_context(tc.tile_pool(name="psden", bufs=1, space="PSUM"))
        ps_misc = ctx.enter_context(tc.tile_pool(name="psmisc", bufs=2, space="PSUM"))

        # ---- PE warm-up: dummy matmuls while the startup DMAs are in
        # flight. The HAM clock gate holds the PE at 1.2 GHz until it sees
        # ~3.4us of sustained activity; burning that window on throwaway
        # matmuls (into the den bank, which is first really used much
        # later) means the first real projections run at full clock.
        warm_sb = const.tile([128, 640], BF16, name="warm_sb")
        nc.vector.memset(warm_sb[:], 0.0)
        warm_ps = ps_den.tile([128, 512], F32, tag="den", name="warm_ps")
        for i in range(10):
            nc.tensor.matmul(
                warm_ps[:], warm_sb[:, 512:640], warm_sb[:, 0:512],
                start=True, stop=True,
            )

        # ---- constants / weights: three DMA rings, big-chunk loads ----
        # Full-row chunks ([128, 1536] for w, [128, 1024] for x) give 3KB/2KB
        # contiguous runs per partition: 2-3x fewer DMA descriptors, so the
        # ring issue time shrinks and the last chunk lands much earlier.
        x_sb = []
        wt_sb = const.tile([128, 4 * 1536], BF16, name="wt_sb")
        rel_sb = const.tile([128, 256], BF16, name="rel_sb")
        sel_sb = const.tile([128, 1024], BF16, name="sel_sb")
        ones_sb = const.tile([128, 1], BF16, name="ones_sb")
        id_sb = const.tile([128, 128], BF16, name="id_sb")

        def _w_row(kc4, eng):
            eng.dma_start(
                wt_sb[:, kc4 * 1536 : kc4 * 1536 + 1536],
                wd[kc4 * 128 : kc4 * 128 + 128, :],
            )

        def _x_row(xb, b, kc4, eng):
            eng.dma_start(
                xb[:, kc4 * HW : kc4 * HW + HW],
                xd[b][kc4 * 128 : kc4 * 128 + 128, :],
            )

        # startup: spread the (w, x) big chunks over the three rings in kc4
        # consumption order so the first projection's operands land
        # earliest; a small disjoint lead pair (head-0 w columns + the
        # first x query-half) cuts the first matmul's wait by ~2us. rel
        # (needed by the first qwh matmuls) leads the gpsimd ring. The
        # scalar ring is only used here, while the ACT queue is still
        # idle — mid-kernel DMA issue on it would stall the exp chain.
        def _wq(kc4, eng):  # q columns only: feed the bias chains
            eng.dma_start(
                wt_sb[:, kc4 * 1536 : kc4 * 1536 + 512],
                wd[kc4 * 128 : kc4 * 128 + 128, 0:512],
            )

        def _wkv(kc4, eng):  # k/v columns: first needed by kv(0), later
            eng.dma_start(
                wt_sb[:, kc4 * 1536 + 512 : kc4 * 1536 + 1536],
                wd[kc4 * 128 : kc4 * 128 + 128, 512:1536],
            )

        xb0 = xpool.tile([128, 4 * HW], BF16, tag="x", name="x_sb0")
        nc.sync.dma_start(xb0[:, 0:512], xd[0][0:128, 0:512])  # x0 lead
        nc.scalar.dma_start(wt_sb[:, 0:128], wd[0:128, 0:128])  # w0-q lead
        nc.gpsimd.dma_start(rel_sb[:], reld)
        nc.scalar.dma_start(wt_sb[:, 128:512], wd[0:128, 128:512])  # w0-q rest
        nc.sync.dma_start(xb0[:, 512:HW], xd[0][0:128, 512:HW])  # x0 rest
        _wq(1, nc.gpsimd)
        _x_row(xb0, 0, 1, nc.scalar)
        _wq(2, nc.sync)
        _x_row(xb0, 0, 2, nc.gpsimd)
        _wq(3, nc.scalar)
        _x_row(xb0, 0, 3, nc.sync)
        x_sb.append(xb0)
        nc.gpsimd.dma_start(id_sb[:], identd)
        _wkv(0, nc.scalar)
        _wkv(1, nc.sync)
        _wkv(2, nc.gpsimd)
        _wkv(3, nc.scalar)
        nc.gpsimd.dma_start(sel_sb[:], seld)
        nc.gpsimd.dma_start(ones_sb[:], onesd)

        def _load_x(b):
            xb = xpool.tile([128, 4 * HW], BF16, tag="x", name=f"x_sb{b}")
            qs = [nc.sync, nc.gpsimd]
            for kc4 in range(4):
                _x_row(xb, b, kc4, qs[kc4 % 2])
            x_sb.append(xb)

        def _proj(bn, qkvT, t, col0):
            b = bn // NH
            for qc in range(2):
                ps = ps_misc.tile(
                    [128, 512], F32, tag="misc", name=f"proj{bn}_{t}_{qc}"
                )
                for kc4 in range(4):
                    nc.tensor.matmul(
                        ps[:],
                        wt_sb[:, kc4 * 1536 + col0 : kc4 * 1536 + col0 + 128],
                        x_sb[b][:, kc4 * HW + qc * 512 : kc4 * HW + qc * 512 + 512],
                        start=(kc4 == 0),
                        stop=(kc4 == 3),
                    )
                # q drain on ACT: q gates the qwh matmuls' weight loads and
                # the DVE queue is long right then. k/v drains on DVE.
                if t == 0:
                    nc.scalar.copy(
                        qkvT[:, t * HW + qc * 512 : t * HW + qc * 512 + 512], ps[:]
                    )
                else:
                    nc.vector.tensor_copy(
                        qkvT[:, t * HW + qc * 512 : t * HW + qc * 512 + 512], ps[:]
                    )

        def stage_bias(bn):
            """q projection + rel tables + skew round trip -> (qkvT, bias_vecT).

            Emitted three bn ahead so the DRAM round trip and the DVE stream
            transposes hide under the previous bns' attention matmuls."""
            n = bn % NH
            if bn == NH:
                _load_x(1)
            scr = scr_handles[bn]
            qkvT = qkvp.tile([128, 3 * HW], BF16, tag="qkv", name=f"qkvT{bn}")
            _proj(bn, qkvT, 0, n * DH)  # q only

            # four N=126 matmuls per PSUM tile (126-wide blocks padded to a
            # 128 stride), drained by one strided DVE copy.
            qwh = qwhp.tile([128, 8 * 126], BF16, tag="qwh", name=f"qwh{bn}")
            for qp4 in range(2):
                ps = ps_misc.tile([128, 512], F32, tag="misc", name=f"qwhp{bn}_{qp4}")
                for j in range(4):
                    nc.tensor.matmul(
                        ps[:, j * 128 : j * 128 + 126],
                        qkvT[:, (qp4 * 4 + j) * 128 : (qp4 * 4 + j) * 128 + 128],
                        rel_sb[:, 0:126],
                        start=True,
                        stop=True,
                    )
                nc.vector.tensor_copy(
                    qwh[:, qp4 * 504 : qp4 * 504 + 504].rearrange(
                        "p (j r) -> p j r", r=126
                    ),
                    ps[:].rearrange("p (j r) -> p j r", j=4)[:, :, 0:126],
                )

            nc.sync.dma_start(
                scr.ap().rearrange("(a p) r -> p a r", p=128),
                qwh[:].rearrange("p (a r) -> p a r", r=126),
            )
            wv = wvp.tile([128, 256], BF16, tag="wv", name=f"wv{bn}")
            hv = wvp.tile([128, 256], BF16, tag="hv", name=f"hv{bn}")
            # split the 8 gather-loads over two queues: serialized on one
            # queue they take ~6us, which starves the bias chain during the
            # pipeline-fill phase.
            for sg in range(4):
                skew_eng = nc.gpsimd if sg % 2 == 0 else nc.sync
                skew_eng.dma_start(
                    wv[32 * sg : 32 * sg + 32, :].rearrange("p (a j) -> p a j", j=32),
                    bass.AP(scr, 31 + 32256 * sg, [[125, 32], [4032, 8], [1, 32]]),
                )
                skew_eng.dma_start(
                    hv[32 * sg : 32 * sg + 32, :].rearrange("p (a j) -> p a j", j=32),
                    bass.AP(scr, 94 + 32255 * sg, [[126, 32], [4028, 8], [1, 32]]),
                )

            # Rows 64..127 multiply zero sel rows but must hold finite
            # values (not stale NaN bit patterns). The transposes only ever
            # write rows 0..63, so a one-time memset per ring buffer is
            # enough — no per-bn duplicate copy.
            bias_vecT = biasp.tile([128, HW], BF16, tag="biasv", name=f"biasv{bn}")
            return qkvT, bias_vecT, wv, hv

        _membn = [0]

        def stage_bias_back(state):
            # The stream transposes are emitted AFTER the next kv stage's
            # k/v drains: they can't start before the skew gathers land
            # anyway, and this keeps the drains ahead of them in the DVE
            # FIFO so the ST/PV stationaries are never stuck behind
            # transpose work.
            _, bias_vecT, wv, hv = state
            for src, row0 in ((wv, 0), (hv, 32)):
                for sg in range(4):
                    nc.vector.transpose(
                        bias_vecT[row0 : row0 + 32, 256 * sg : 256 * sg + 256],
                        src[32 * sg : 32 * sg + 32, :],
                    )
            # one-time per ring buffer: rows 64..127 multiply zero sel rows
            # but must be finite. Buffers 0-2 are memset in one batch at
            # warmup end (so they never delay the startup bias chains);
            # 3-4 here, after the transposes (disjoint rows), mid-kernel.
            if 3 <= _membn[0] < BIAS_BUFS:
                nc.vector.memset(bias_vecT[64:128, :], 0.0)
            _membn[0] += 1

        def stage_kv(bn, qkvT):
            n = bn % NH
            _proj(bn, qkvT, 1, 512 + n * DH)  # k
            _proj(bn, qkvT, 2, 1024 + n * DH)  # v
            vnat = vnatp.tile([128, HW], BF16, tag="vnat", name=f"vnat{bn}")
            # batch 4 transposes per PSUM tile so one wide copy drains them:
            # fewer DVE instructions and fewer ldweights waits on the drain.
            for half in range(2):
                ps = ps_misc.tile([128, 512], BF16, tag="misc", name=f"vtr{bn}_{half}")
                for j in range(4):
                    kc = half * 4 + j
                    nc.tensor.transpose(
                        ps[:, j * 128 : j * 128 + 128],
                        qkvT[:, 2 * HW + kc * 128 : 2 * HW + kc * 128 + 128],
                        id_sb[:],
                    )
                # drain on ACT: these copies gate the next bn's projection
                # PSUM slots, and the DVE queue is long right here.
                nc.scalar.copy(vnat[:, half * 512 : half * 512 + 512], ps[:])
            return vnat

        def stage_attn(bn, qkvT, bias_vecT, vnat):
            b, n = bn // NH, bn % NH

            out_ps = [
                ps_out.tile([128, 512], F32, tag="out", name=f"outp{bn}_{i}")
                for i in range(2)
            ]

            def emit_st(kc):
                pT = ptp.tile([128, HW], BF16, tag="pt", name=f"pt{bn}_{kc}")
                for qc in range(2):
                    st = ps_st.tile([128, 512], F32, tag="st", name=f"st{bn}_{kc}_{qc}")
                    nc.tensor.matmul(
                        st[:],
                        qkvT[:, HW + kc * 128 : HW + kc * 128 + 128],
                        qkvT[:, qc * 512 : qc * 512 + 512],
                        start=True,
                        stop=False,
                    )
                    nc.tensor.matmul(
                        st[:],
                        sel_sb[:, kc * 128 : kc * 128 + 128],
                        bias_vecT[:, qc * 512 : qc * 512 + 512],
                        start=False,
                        stop=True,
                    )
                    nc.scalar.activation(
                        pT[:, qc * 512 : qc * 512 + 512], st[:], EXP
                    )
                return pT

            def emit_pv(kc, pT):
                for qc in range(2):
                    nc.tensor.matmul(
                        out_ps[qc][:],
                        vnat[:, kc * 128 : kc * 128 + 128],
                        pT[:, qc * 512 : qc * 512 + 512],
                        start=(kc == 0),
                        stop=(kc == 7),
                    )

            # The M=1 den matmuls run in a different PE column-group config
            # than everything else; each entry/exit costs ~95ns. So the kc
            # loop interleaves only the (config-identical) PV matmuls at a
            # 3-tile lookahead, and ALL den matmuls run as one contiguous
            # block at the end of the head.
            last = bn == B_LOC * NH - 1
            from collections import deque as _dq

            pend = _dq()
            pts = []
            # final bn: shallow lookahead — hides the exp latency while
            # keeping the drain tail short.
            depth = 1 if last else 4
            for kc in range(8):
                pT = emit_st(kc)
                pts.append(pT)
                pend.append((kc, pT))
                if len(pend) > depth:
                    kc2, p2 = pend.popleft()
                    emit_pv(kc2, p2)
            def emit_den():
                # Denominators: M=1 ones-stationary matmuls column-tiled
                # over the four 32-wide PE column groups, so each wave of 4
                # runs concurrently. Position 32*(2*qc+half) accumulates
                # k-tiles 4*half + (0..3) for query half qc; partials land
                # at PSUM partitions 0/32/64/96 and are summed on the host.
                den_all = ps_den.tile([128, 512], F32, tag="den", name=f"den{bn}")
                for w in range(4):
                    for qc in range(2):
                        for half in range(2):
                            kc = 4 * half + w
                            pos = 32 * (2 * qc + half)
                            nc.tensor.matmul(
                                den_all[pos : pos + 1, :],
                                ones_sb[:, 0:1],
                                pts[kc][:, qc * 512 : qc * 512 + 512],
                                start=(w == 0),
                                stop=(w == 3),
                                tile_position=(0, pos),
                            )
                return den_all

            if last:
                # den first so its drain + store DMA overlap the final PVs
                den_all = emit_den()
                while pend:
                    kc2, p2 = pend.popleft()
                    emit_pv(kc2, p2)
            else:
                while pend:
                    kc2, p2 = pend.popleft()
                    emit_pv(kc2, p2)
                den_all = emit_den()

            outT = outp.tile([128, HW], F32, tag="outt", name=f"outT{bn}")
            den_sb = recp.tile([128, 512], F32, tag="densb", name=f"densb{bn}")
            nc.vector.tensor_copy(den_sb[0:97, :], den_all[0:97, :])
            # the sync ring's DMA-completion drain is ~1.5us faster than
            # gpsimd's, so the final head's stores all go to sync
            den_ring = nc.sync if last else nc.gpsimd
            den_ring.dma_start(
                dend[b, n].rearrange("a (o f) -> a o f", o=1),
                den_sb[:].rearrange("(a p) f -> a p f", p=32)[:, 0:1, :],
            )
            for qc in range(2):
                # outT drains on ACT: they gate the next head's first PV
                # (PSUM out-bank reuse) and the DVE queue is busy with the
                # pipelined bias transposes right here. For the LAST heads
                # there are no more bias/kv stages — the exp chain is the
                # pacer and DVE has slack, so split the two drains across
                # DVE and ACT to run them in parallel.
                if bn >= B_LOC * NH - 3 and qc == 0:
                    nc.vector.tensor_copy(
                        outT[:, qc * 512 : qc * 512 + 512], out_ps[qc][:]
                    )
                else:
                    nc.scalar.copy(
                        outT[:, qc * 512 : qc * 512 + 512], out_ps[qc][:]
                    )
                # kick the output DMA per q-half so the last store overlaps
                nc.sync.dma_start(
                    outd[b, n][:, qc * 512 : qc * 512 + 512],
                    outT[:, qc * 512 : qc * 512 + 512],
                )

        # software pipeline: bias chains emitted three bn ahead of attention
        n_bn = B_LOC * NH
        states = {}
        kvs = {}
        states[0] = stage_bias(0)
        states[1] = stage_bias(1)
        stage_bias_back(states[0])
        kvs[0] = stage_kv(0, states[0][0])
        stage_bias_back(states[1])
        states[2] = stage_bias(2)
        kvs[1] = stage_kv(1, states[1][0])
        stage_bias_back(states[2])
        # one-time finite-fill of the first three bias ring buffers' zero
        # rows — batched here so the DVE handles them only after the
        # startup bias chains' transposes, still before the first sel use
        for i in range(3):
            nc.vector.memset(states[i][1][64:128, :], 0.0)
        for bn in range(n_bn):
            if bn + 3 < n_bn:
                states[bn + 3] = stage_bias(bn + 3)
            if bn + 2 < n_bn:
                kvs[bn + 2] = stage_kv(bn + 2, states[bn + 2][0])
            if bn + 3 < n_bn:
                stage_bias_back(states[bn + 3])
            qkvT, bias_vecT = states.pop(bn)[0:2]
            stage_attn(bn, qkvT, bias_vecT, kvs.pop(bn))


def _build():
    if "nc" in _CACHE:
        return _CACHE["nc"]
    nc = bacc.Bacc("TRN2", target_bir_lowering=False, debug=False, num_devices=N_CORES)
    xd = nc.dram_tensor("x_r", [B_LOC, C, HW], BF16, kind="ExternalInput").ap()
    wd = nc.dram_tensor("w_t", [C, 3 * NH * DH], BF16, kind="ExternalInput").ap()
    reld = nc.dram_tensor("rel_t", [128, 256], BF16, kind="ExternalInput").ap()
    seld = nc.dram_tensor("sel", [128, 1024], BF16, kind="ExternalInput").ap()
    onesd = nc.dram_tensor("ones", [128, 1], BF16, kind="ExternalInput").ap()
    identd = nc.dram_tensor("ident", [128, 128], BF16, kind="ExternalInput").ap()
    outd = nc.dram_tensor("out_r", [B_LOC, NH, DH, HW], F32, kind="ExternalOutput").ap()
    dend = nc.dram_tensor("den_r", [B_LOC, NH, 4, 512], F32, kind="ExternalOutput").ap()
    scr_handles = [
        nc.dram_tensor(f"scr{i}", [HW, 126], BF16) for i in range(B_LOC * NH)
    ]
    with tile.TileContext(nc) as tc:
        _emit(tc, nc, xd, wd, reld, seld, onesd, identd, outd, dend, scr_handles)
    nc.compile()
    _CACHE["nc"] = nc
    return nc


def _in_maps(x, w_qkv, height_rel, width_rel):
    x = np.asarray(x, np.float32)
    w_qkv = np.asarray(w_qkv, np.float32)
    height_rel = np.asarray(height_rel, np.float32)
    width_rel = np.asarray(width_rel, np.float32)

    w_t = np.ascontiguousarray(w_qkv.T)  # [C, 1536]
    w_t[:, 512:1024] *= np.float32(SCALE)  # fold softmax scale into k
    w_t = w_t.astype(ml_dtypes.bfloat16)
    rel_t = np.zeros((128, 256), np.float32)
    rel_t[:, 0:63] = width_rel.T
    rel_t[:, 63:126] = height_rel.T
    rel_t = rel_t.astype(ml_dtypes.bfloat16)
    sel = _sel_const().astype(ml_dtypes.bfloat16)
    ones = np.ones((128, 1), np.float32).astype(ml_dtypes.bfloat16)
    ident = np.eye(128, dtype=np.float32).astype(ml_dtypes.bfloat16)

    # pre-swizzle x's hw columns into device f-order
    xf = x.reshape(B, C, HW)[:, :, _Q_OF_F].astype(ml_dtypes.bfloat16)

    shared = {
        "w_t": w_t,
        "rel_t": rel_t,
        "sel": sel,
        "ones": ones,
        "ident": ident,
    }
    maps = []
    for i in range(N_CORES):
        xm = xf[i * B_LOC : (i + 1) * B_LOC]
        maps.append({"x_r": np.ascontiguousarray(xm), **shared})
    return maps


def _assemble(results):
    out = np.empty((B, 3 * NH * DH // 3, H, W), np.float32)  # (16, 512, 32, 32)
    for i, r in enumerate(results):
        den4 = r["den_r"]  # [B_LOC, NH, 4, 512]: (qc0 lo, qc0 hi, qc1 lo, qc1 hi)
        den = np.concatenate(
            [den4[:, :, 0] + den4[:, :, 1], den4[:, :, 2] + den4[:, :, 3]], axis=-1
        )  # [B_LOC, NH, 1024]
        arr = r["out_r"] / den[:, :, None, :]  # [B_LOC, NH, DH, HW]
        arr = arr[..., _F_OF_Q]  # undo the device-side f-ordering of hw columns
        for b in range(B_LOC):
            # flat order of reference output = q*512 + n*128 + d
            out[i * B_LOC + b] = (
                arr[b].transpose(2, 0, 1).reshape(512, 32, 32)
            )
    return out


def run(x, w_qkv, height_rel, width_rel, **spmd_kwargs):
    nc = _build()
    maps = _in_maps(x, w_qkv, height_rel, width_rel)
    res = run_bass_kernel_spmd(nc, maps, core_ids=list(range(N_CORES)), **spmd_kwargs)
    return _assemble(res.results), res


def kernel(x, w_qkv, height_rel, width_rel):
    out, _ = run(x, w_qkv, height_rel, width_rel)
    return out


# revision 37
# speedup vs baseline: 1.0057x; 1.0057x over previous
"""BottleneckAttn TRN2 kernel.

Reference computation (per batch b, head n, fp32):
    qkv = w_qkv @ x_b                      # (1536, 1024), 1x1 conv
    q, k, v per head: (1024, 128) with hw = h*32 + w
    logits[q,k] = SCALE * (q . k) + qw[q, 31 + w2(k) - w(q)] + qh[q, 31 + h2(k) - h(q)]
        where qw[q,r] = q . width_rel[r], qh[q,r] = q . height_rel[r]
    out = softmax(logits) @ v              # (1024, 128)
    output[b] flat index = q*512 + n*128 + d  -> reshape (512, 32, 32)

Device strategy (SPMD, 8 cores, 2 batches/core):
  - All matmuls computed in the TRANSPOSED softmax layout ST[k, q] so the
    attention probabilities come out of the exp directly in the layout the
    PV matmul needs as its moving operand (no P transposes).
  - ST = k @ qT via PE (operands swapped); the relative-position bias is
    folded in as a second accumulating matmul with a constant 0/1 selection
    matrix lhsT (rows select the shifted width/height tables per PSUM
    partition).
  - The per-query shifted tables (skew gather) are built by a DRAM round
    trip: qw/qh computed in [q, r] layout on PE, stored to a DRAM scratch,
    re-loaded with an affine skewed access pattern (contiguous 32-element
    runs), then rotated into [table_row, q] layout with DVE 32x32 stream
    transposes.
  - Softmax denominators via an all-ones stationary matmul with M=1 outputs
    COLUMN-TILED over the four 32-wide PE column groups (tile_position):
    4 concurrent denominator matmuls per wave, 4 waves per head, partials
    at PSUM partitions 0/32/64/96 summed on the host after the gather.
  - EVERY matmul runs bf16: measured fastest issue rate (~216ns per
    512-col matmul vs 227ns f32r), but ONLY when the PE instruction
    stream is dtype-homogeneous — mixing fp16/f32r modes costs ~100ns
    per switch. The sel stationary is zero-padded to K=128: the 64-row
    (row_grp) PE configuration measured ~2x the issue interval.
  - PV matmuls for k-tile kc are emitted after the ST+bias matmuls of
    kc+1 so the scalar-engine exp has a full k-tile of slack to hide in.
  - The bias chain (q proj + qwh + DRAM skew trip + DVE transposes) is
    pipelined THREE heads ahead of attention so the DVE transpose queue
    never blocks the PE's sel matmuls.
  - x is pre-swizzled to the device's f-order hw columns on the host, so
    the projection matmuls read plain contiguous slices.
  - Startup loads are spread over the three DMA rings (sync/scalar/gpsimd)
    interleaved in first-use order.
"""

import os
import sys

import numpy as np

for _p in ("/opt/trn_rl_repo", "/root/.axon_site/_ro/trn_rl_repo"):
    if os.path.isdir(_p) and _p not in sys.path:
        sys.path.append(_p)

import ml_dtypes

import concourse.bass as bass
import concourse.mybir as mybir
import concourse.tile as tile
from concourse import bacc
from concourse.bass_utils import run_bass_kernel_spmd

B, C, H, W = 16, 512, 32, 32
HW = H * W
NH, DH = 4, 128
SCALE = DH ** -0.5
N_CORES = 8
B_LOC = B // N_CORES

F32 = mybir.dt.float32
F32R = mybir.dt.float32r
BF16 = mybir.dt.bfloat16
EXP = mybir.ActivationFunctionType.Exp

_CACHE = {}

# f-order permutation: device hw column f = 256*s + 32*qt + j for q = 128*qt + 32*s + j
_QS = np.arange(1024)
_F_OF_Q = (256 * ((_QS % 128) // 32) + 32 * (_QS // 128) + (_QS % 32)).astype(np.int64)
_Q_OF_F = np.argsort(_F_OF_Q)

BIAS_BUFS = 5  # bias_vecT ring size (pipeline depth 3 + slack)


def _sel_const():
    """sel[j, kc*128 + p]: j<32 selects shifted width row w2(p), j in
    [32,64) selects shifted height row h2(p). Rows 64..127 are ZERO
    padding: a full-height K=128 stationary avoids the slow 64-row
    (row_grp) PE configuration measured at ~2x the issue interval."""
    sel = np.zeros((128, 8 * 128), np.float32)
    for kc in range(8):
        for p in range(128):
            f = 128 * kc + p
            sg, qt, j = f // 256, (f % 256) // 32, f % 32
            q = 128 * qt + 32 * sg + j
            sel[q % 32, kc * 128 + p] = 1.0
            sel[32 + q // 32, kc * 128 + p] = 1.0
    return sel  # [128, 1024]


def _emit(tc, nc, xd, wd, reld, seld, onesd, identd, outd, dend, scr_handles):
    from contextlib import ExitStack

    ctx = ExitStack()
    with ctx:
        const = ctx.enter_context(tc.tile_pool(name="const", bufs=1))
        xpool = ctx.enter_context(tc.tile_pool(name="x", bufs=B_LOC))
        qkvp = ctx.enter_context(tc.tile_pool(name="qkv", bufs=BIAS_BUFS))
        qwhp = ctx.enter_context(tc.tile_pool(name="qwh", bufs=4))
        wvp = ctx.enter_context(tc.tile_pool(name="wv", bufs=4))
        biasp = ctx.enter_context(tc.tile_pool(name="biasv", bufs=BIAS_BUFS))
        vnatp = ctx.enter_context(tc.tile_pool(name="vnat", bufs=2))
        ptp = ctx.enter_context(tc.tile_pool(name="pt", bufs=12))
        outp = ctx.enter_context(tc.tile_pool(name="outt", bufs=2))
        recp = ctx.enter_context(tc.tile_pool(name="recip", bufs=2))
        # PSUM budget (8 banks): st 2 + out 2 + den 1 + misc 3
        ps_st = ctx.enter_context(tc.tile_pool(name="psst", bufs=2, space="PSUM"))
        ps_out = ctx.enter_context(tc.tile_pool(name="psout", bufs=2, space="PSUM"))
        ps_den = ctx.enter_context(tc.tile_pool(name="psden", bufs=1, space="PSUM"))
        ps_misc = ctx.enter_context(tc.tile_pool(name="psmisc", bufs=3, space="PSUM"))

        # ---- PE warm-up: dummy matmuls while the startup DMAs are in
        # flight. The HAM clock gate holds the PE at 1.2 GHz until it sees
        # ~3.4us of sustained activity; burning that window on throwaway
        # matmuls (into the den bank, which is first really used much
        # later) means the first real projections run at full clock.
        warm_sb = const.tile([128, 640], BF16, name="warm_sb")
        nc.vector.memset(warm_sb[:], 0.0)
        warm_ps = ps_den.tile([128, 512], F32, tag="den", name="warm_ps")
        for i in range(10):
            nc.tensor.matmul(
                warm_ps[:], warm_sb[:, 512:640], warm_sb[:, 0:512],
                start=True, stop=True,
            )

        # ---- constants / weights: three DMA rings, big-chunk loads ----
        # Full-row chunks ([128, 1536] for w, [128, 1024] for x) give 3KB/2KB
        # contiguous runs per partition: 2-3x fewer DMA descriptors, so the
        # ring issue time shrinks and the last chunk lands much earlier.
        x_sb = []
        wt_sb = const.tile([128, 4 * 1536], BF16, name="wt_sb")
        rel_sb = const.tile([128, 256], BF16, name="rel_sb")
        sel_sb = const.tile([128, 1024], BF16, name="sel_sb")
        ones_sb = const.tile([128, 1], BF16, name="ones_sb")
        id_sb = const.tile([128, 128], BF16, name="id_sb")

        def _w_row(kc4, eng):
            eng.dma_start(
                wt_sb[:, kc4 * 1536 : kc4 * 1536 + 1536],
                wd[kc4 * 128 : kc4 * 128 + 128, :],
            )

        def _x_row(xb, b, kc4, eng):
            eng.dma_start(
                xb[:, kc4 * HW : kc4 * HW + HW],
                xd[b][kc4 * 128 : kc4 * 128 + 128, :],
            )

        # startup: spread the (w, x) big chunks over the three rings in kc4
        # consumption order so the first projection's operands land
        # earliest; a small disjoint lead pair (head-0 w columns + the
        # first x query-half) cuts the first matmul's wait by ~2us. rel
        # (needed by the first qwh matmuls) leads the gpsimd ring. The
        # scalar ring is only used here, while the ACT queue is still
        # idle — mid-kernel DMA issue on it would stall the exp chain.
        def _wq(kc4, eng):  # q columns only: feed the bias chains
            eng.dma_start(
                wt_sb[:, kc4 * 1536 : kc4 * 1536 + 512],
                wd[kc4 * 128 : kc4 * 128 + 128, 0:512],
            )

        def _wkv(kc4, eng):  # k/v columns: first needed by kv(0), later
            eng.dma_start(
                wt_sb[:, kc4 * 1536 + 512 : kc4 * 1536 + 1536],
                wd[kc4 * 128 : kc4 * 128 + 128, 512:1536],
            )

        xb0 = xpool.tile([128, 4 * HW], BF16, tag="x", name="x_sb0")
        nc.sync.dma_start(xb0[:, 0:512], xd[0][0:128, 0:512])  # x0 lead
        nc.scalar.dma_start(wt_sb[:, 0:128], wd[0:128, 0:128])  # w0-q lead
        nc.gpsimd.dma_start(rel_sb[:], reld)
        nc.scalar.dma_start(wt_sb[:, 128:512], wd[0:128, 128:512])  # w0-q rest
        nc.sync.dma_start(xb0[:, 512:HW], xd[0][0:128, 512:HW])  # x0 rest
        _wq(1, nc.gpsimd)
        _x_row(xb0, 0, 1, nc.scalar)
        _wq(2, nc.sync)
        _x_row(xb0, 0, 2, nc.gpsimd)
        _wq(3, nc.scalar)
        _x_row(xb0, 0, 3, nc.sync)
        x_sb.append(xb0)
        nc.gpsimd.dma_start(id_sb[:], identd)
        _wkv(0, nc.scalar)
        _wkv(1, nc.sync)
        _wkv(2, nc.gpsimd)
        _wkv(3, nc.scalar)
        nc.gpsimd.dma_start(sel_sb[:], seld)
        nc.gpsimd.dma_start(ones_sb[:], onesd)

        def _load_x(b):
            xb = xpool.tile([128, 4 * HW], BF16, tag="x", name=f"x_sb{b}")
            qs = [nc.sync, nc.gpsimd]
            for kc4 in range(4):
                _x_row(xb, b, kc4, qs[kc4 % 2])
            x_sb.append(xb)

        def _proj(bn, qkvT, t, col0):
            b = bn // NH
            for qc in range(2):
                ps = ps_misc.tile(
                    [128, 512], F32, tag="misc", name=f"proj{bn}_{t}_{qc}"
                )
                for kc4 in range(4):
                    nc.tensor.matmul(
                        ps[:],
                        wt_sb[:, kc4 * 1536 + col0 : kc4 * 1536 + col0 + 128],
                        x_sb[b][:, kc4 * HW + qc * 512 : kc4 * HW + qc * 512 + 512],
                        start=(kc4 == 0),
                        stop=(kc4 == 3),
                    )
                # q drain on ACT: q gates the qwh matmuls' weight loads and
                # the DVE queue is long right then. k/v drains on DVE.
                if t == 0:
                    nc.scalar.copy(
                        qkvT[:, t * HW + qc * 512 : t * HW + qc * 512 + 512], ps[:]
                    )
                else:
                    nc.vector.tensor_copy(
                        qkvT[:, t * HW + qc * 512 : t * HW + qc * 512 + 512], ps[:]
                    )

        def stage_bias(bn):
            """q projection + rel tables + skew round trip -> (qkvT, bias_vecT).

            Emitted three bn ahead so the DRAM round trip and the DVE stream
            transposes hide under the previous bns' attention matmuls."""
            n = bn % NH
            if bn == NH:
                _load_x(1)
            scr = scr_handles[bn]
            qkvT = qkvp.tile([128, 3 * HW], BF16, tag="qkv", name=f"qkvT{bn}")
            _proj(bn, qkvT, 0, n * DH)  # q only

            # four N=126 matmuls per PSUM tile (126-wide blocks padded to a
            # 128 stride), drained by one strided DVE copy.
            qwh = qwhp.tile([128, 8 * 126], BF16, tag="qwh", name=f"qwh{bn}")
            for qp4 in range(2):
                ps = ps_misc.tile([128, 512], F32, tag="misc", name=f"qwhp{bn}_{qp4}")
                for j in range(4):
                    nc.tensor.matmul(
                        ps[:, j * 128 : j * 128 + 126],
                        qkvT[:, (qp4 * 4 + j) * 128 : (qp4 * 4 + j) * 128 + 128],
                        rel_sb[:, 0:126],
                        start=True,
                        stop=True,
                    )
                nc.vector.tensor_copy(
                    qwh[:, qp4 * 504 : qp4 * 504 + 504].rearrange(
                        "p (j r) -> p j r", r=126
                    ),
                    ps[:].rearrange("p (j r) -> p j r", j=4)[:, :, 0:126],
                )

            nc.sync.dma_start(
                scr.ap().rearrange("(a p) r -> p a r", p=128),
                qwh[:].rearrange("p (a r) -> p a r", r=126),
            )
            wv = wvp.tile([128, 256], BF16, tag="wv", name=f"wv{bn}")
            hv = wvp.tile([128, 256], BF16, tag="hv", name=f"hv{bn}")
            # split the 8 gather-loads over two queues: serialized on one
            # queue they take ~6us, which starves the bias chain during the
            # pipeline-fill phase.
            for sg in range(4):
                skew_eng = nc.gpsimd if sg % 2 == 0 else nc.sync
                skew_eng.dma_start(
                    wv[32 * sg : 32 * sg + 32, :].rearrange("p (a j) -> p a j", j=32),
                    bass.AP(scr, 31 + 32256 * sg, [[125, 32], [4032, 8], [1, 32]]),
                )
                skew_eng.dma_start(
                    hv[32 * sg : 32 * sg + 32, :].rearrange("p (a j) -> p a j", j=32),
                    bass.AP(scr, 94 + 32255 * sg, [[126, 32], [4028, 8], [1, 32]]),
                )

            # Rows 64..127 multiply zero sel rows but must hold finite
            # values (not stale NaN bit patterns). The transposes only ever
            # write rows 0..63, so a one-time memset per ring buffer is
            # enough — no per-bn duplicate copy.
            bias_vecT = biasp.tile([128, HW], BF16, tag="biasv", name=f"biasv{bn}")
            return qkvT, bias_vecT, wv, hv

        _membn = [0]

        def stage_bias_back(state):
            # The stream transposes are emitted AFTER the next kv stage's
            # k/v drains: they can't start before the skew gathers land
            # anyway, and this keeps the drains ahead of them in the DVE
            # FIFO so the ST/PV stationaries are never stuck behind
            # transpose work.
            _, bias_vecT, wv, hv = state
            for src, row0 in ((wv, 0), (hv, 32)):
                for sg in range(4):
                    nc.vector.transpose(
                        bias_vecT[row0 : row0 + 32, 256 * sg : 256 * sg + 256],
                        src[32 * sg : 32 * sg + 32, :],
                    )
            # one-time per ring buffer: rows 64..127 multiply zero sel rows
            # but must be finite. Buffers 0-2 are memset in one batch at
            # warmup end (so they never delay the startup bias chains);
            # 3-4 here, after the transposes (disjoint rows), mid-kernel.
            if 3 <= _membn[0] < BIAS_BUFS:
                nc.vector.memset(bias_vecT[64:128, :], 0.0)
            _membn[0] += 1

        def stage_kv(bn, qkvT):
            n = bn % NH
            _proj(bn, qkvT, 1, 512 + n * DH)  # k
            _proj(bn, qkvT, 2, 1024 + n * DH)  # v
            vnat = vnatp.tile([128, HW], BF16, tag="vnat", name=f"vnat{bn}")
            # batch 4 transposes per PSUM tile so one wide copy drains them:
            # fewer DVE instructions and fewer ldweights waits on the drain.
            for half in range(2):
                ps = ps_misc.tile([128, 512], BF16, tag="misc", name=f"vtr{bn}_{half}")
                for j in range(4):
                    kc = half * 4 + j
                    nc.tensor.transpose(
                        ps[:, j * 128 : j * 128 + 128],
                        qkvT[:, 2 * HW + kc * 128 : 2 * HW + kc * 128 + 128],
                        id_sb[:],
                    )
                # drain on ACT: these copies gate the next bn's projection
                # PSUM slots, and the DVE queue is long right here.
                nc.scalar.copy(vnat[:, half * 512 : half * 512 + 512], ps[:])
            return vnat

        def stage_attn(bn, qkvT, bias_vecT, vnat):
            b, n = bn // NH, bn % NH

            out_ps = [
                ps_out.tile([128, 512], F32, tag="out", name=f"outp{bn}_{i}")
                for i in range(2)
            ]

            def emit_st(kc):
                pT = ptp.tile([128, HW], BF16, tag="pt", name=f"pt{bn}_{kc}")
                for qc in range(2):
                    st = ps_st.tile([128, 512], F32, tag="st", name=f"st{bn}_{kc}_{qc}")
                    nc.tensor.matmul(
                        st[:],
                        qkvT[:, HW + kc * 128 : HW + kc * 128 + 128],
                        qkvT[:, qc * 512 : qc * 512 + 512],
                        start=True,
                        stop=False,
                    )
                    nc.tensor.matmul(
                        st[:],
                        sel_sb[:, kc * 128 : kc * 128 + 128],
                        bias_vecT[:, qc * 512 : qc * 512 + 512],
                        start=False,
                        stop=True,
                    )
                    nc.scalar.activation(
                        pT[:, qc * 512 : qc * 512 + 512], st[:], EXP
                    )
                return pT

            def emit_pv(kc, pT):
                for qc in range(2):
                    nc.tensor.matmul(
                        out_ps[qc][:],
                        vnat[:, kc * 128 : kc * 128 + 128],
                        pT[:, qc * 512 : qc * 512 + 512],
                        start=(kc == 0),
                        stop=(kc == 7),
                    )

            # The M=1 den matmuls run in a different PE column-group config
            # than everything else; each entry/exit costs ~95ns. So the kc
            # loop interleaves only the (config-identical) PV matmuls at a
            # 3-tile lookahead, and ALL den matmuls run as one contiguous
            # block at the end of the head.
            last = bn == B_LOC * NH - 1
            from collections import deque as _dq

            pend = _dq()
            pts = []
            # final bn: shallow lookahead — hides the exp latency while
            # keeping the drain tail short.
            depth = 1 if last else 4
            for kc in range(8):
                pT = emit_st(kc)
                pts.append(pT)
                pend.append((kc, pT))
                if len(pend) > depth:
                    kc2, p2 = pend.popleft()
                    emit_pv(kc2, p2)
            def emit_den():
                # Denominators: M=1 ones-stationary matmuls column-tiled
                # over the four 32-wide PE column groups, so each wave of 4
                # runs concurrently. Position 32*(2*qc+half) accumulates
                # k-tiles 4*half + (0..3) for query half qc; partials land
                # at PSUM partitions 0/32/64/96 and are summed on the host.
                den_all = ps_den.tile([128, 512], F32, tag="den", name=f"den{bn}")
                for w in range(4):
                    for qc in range(2):
                        for half in range(2):
                            kc = 4 * half + w
                            pos = 32 * (2 * qc + half)
                            nc.tensor.matmul(
                                den_all[pos : pos + 1, :],
                                ones_sb[:, 0:1],
                                pts[kc][:, qc * 512 : qc * 512 + 512],
                                start=(w == 0),
                                stop=(w == 3),
                                tile_position=(0, pos),
                            )
                return den_all

            if last:
                # den first so its drain + store DMA overlap the final PVs
                den_all = emit_den()
                while pend:
                    kc2, p2 = pend.popleft()
                    emit_pv(kc2, p2)
            else:
                while pend:
                    kc2, p2 = pend.popleft()
                    emit_pv(kc2, p2)
                den_all = emit_den()

            outT = outp.tile([128, HW], F32, tag="outt", name=f"outT{bn}")
            den_sb = recp.tile([128, 512], F32, tag="densb", name=f"densb{bn}")
            nc.vector.tensor_copy(den_sb[0:97, :], den_all[0:97, :])
            # the sync ring's DMA-completion drain is ~1.5us faster than
            # gpsimd's, so the final head's stores all go to sync
            den_ring = nc.sync if last else nc.gpsimd
            den_ring.dma_start(
                dend[b, n].rearrange("a (o f) -> a o f", o=1),
                den_sb[:].rearrange("(a p) f -> a p f", p=32)[:, 0:1, :],
            )
            for qc in range(2):
                # outT drains on ACT: they gate the next head's first PV
                # (PSUM out-bank reuse) and the DVE queue is busy with the
                # pipelined bias transposes right here. For the LAST heads
                # there are no more bias/kv stages — the exp chain is the
                # pacer and DVE has slack, so split the two drains across
                # DVE and ACT to run them in parallel.
                if bn >= B_LOC * NH - 3 and qc == 0:
                    nc.vector.tensor_copy(
                        outT[:, qc * 512 : qc * 512 + 512], out_ps[qc][:]
                    )
                else:
                    nc.scalar.copy(
                        outT[:, qc * 512 : qc * 512 + 512], out_ps[qc][:]
                    )
                # kick the output DMA per q-half so the last store overlaps
                nc.sync.dma_start(
                    outd[b, n][:, qc * 512 : qc * 512 + 512],
                    outT[:, qc * 512 : qc * 512 + 512],
                )

        # software pipeline: bias chains emitted three bn ahead of attention
        n_bn = B_LOC * NH
        states = {}
        kvs = {}
        states[0] = stage_bias(0)
        states[1] = stage_bias(1)
        stage_bias_back(states[0])
        kvs[0] = stage_kv(0, states[0][0])
        stage_bias_back(states[1])
        states[2] = stage_bias(2)
        kvs[1] = stage_kv(1, states[1][0])
        stage_bias_back(states[2])
        # one-time finite-fill of the first three bias ring buffers' zero
        # rows — batched here so the DVE handles them only after the
        # startup bias chains' transposes, still before the first sel use
        for i in range(3):
            nc.vector.memset(states[i][1][64:128, :], 0.0)
        for bn in range(n_bn):
            if bn + 3 < n_bn:
                states[bn + 3] = stage_bias(bn + 3)
            if bn + 2 < n_bn:
                kvs[bn + 2] = stage_kv(bn + 2, states[bn + 2][0])
            if bn + 3 < n_bn:
                stage_bias_back(states[bn + 3])
            qkvT, bias_vecT = states.pop(bn)[0:2]
            stage_attn(bn, qkvT, bias_vecT, kvs.pop(bn))


def _build():
    if "nc" in _CACHE:
        return _CACHE["nc"]
    nc = bacc.Bacc("TRN2", target_bir_lowering=False, debug=False, num_devices=N_CORES)
    xd = nc.dram_tensor("x_r", [B_LOC, C, HW], BF16, kind="ExternalInput").ap()
    wd = nc.dram_tensor("w_t", [C, 3 * NH * DH], BF16, kind="ExternalInput").ap()
    reld = nc.dram_tensor("rel_t", [128, 256], BF16, kind="ExternalInput").ap()
    seld = nc.dram_tensor("sel", [128, 1024], BF16, kind="ExternalInput").ap()
    onesd = nc.dram_tensor("ones", [128, 1], BF16, kind="ExternalInput").ap()
    identd = nc.dram_tensor("ident", [128, 128], BF16, kind="ExternalInput").ap()
    outd = nc.dram_tensor("out_r", [B_LOC, NH, DH, HW], F32, kind="ExternalOutput").ap()
    dend = nc.dram_tensor("den_r", [B_LOC, NH, 4, 512], F32, kind="ExternalOutput").ap()
    scr_handles = [
        nc.dram_tensor(f"scr{i}", [HW, 126], BF16) for i in range(B_LOC * NH)
    ]
    with tile.TileContext(nc) as tc:
        _emit(tc, nc, xd, wd, reld, seld, onesd, identd, outd, dend, scr_handles)
    nc.compile()
    _CACHE["nc"] = nc
    return nc


def _in_maps(x, w_qkv, height_rel, width_rel):
    x = np.asarray(x, np.float32)
    w_qkv = np.asarray(w_qkv, np.float32)
    height_rel = np.asarray(height_rel, np.float32)
    width_rel = np.asarray(width_rel, np.float32)

    w_t = np.ascontiguousarray(w_qkv.T)  # [C, 1536]
    w_t[:, 512:1024] *= np.float32(SCALE)  # fold softmax scale into k
    w_t = w_t.astype(ml_dtypes.bfloat16)
    rel_t = np.zeros((128, 256), np.float32)
    rel_t[:, 0:63] = width_rel.T
    rel_t[:, 63:126] = height_rel.T
    rel_t = rel_t.astype(ml_dtypes.bfloat16)
    sel = _sel_const().astype(ml_dtypes.bfloat16)
    ones = np.ones((128, 1), np.float32).astype(ml_dtypes.bfloat16)
    ident = np.eye(128, dtype=np.float32).astype(ml_dtypes.bfloat16)

    # pre-swizzle x's hw columns into device f-order
    xf = x.reshape(B, C, HW)[:, :, _Q_OF_F].astype(ml_dtypes.bfloat16)

    shared = {
        "w_t": w_t,
        "rel_t": rel_t,
        "sel": sel,
        "ones": ones,
        "ident": ident,
    }
    maps = []
    for i in range(N_CORES):
        xm = xf[i * B_LOC : (i + 1) * B_LOC]
        maps.append({"x_r": np.ascontiguousarray(xm), **shared})
    return maps


def _assemble(results):
    out = np.empty((B, 3 * NH * DH // 3, H, W), np.float32)  # (16, 512, 32, 32)
    for i, r in enumerate(results):
        den4 = r["den_r"]  # [B_LOC, NH, 4, 512]: (qc0 lo, qc0 hi, qc1 lo, qc1 hi)
        den = np.concatenate(
            [den4[:, :, 0] + den4[:, :, 1], den4[:, :, 2] + den4[:, :, 3]], axis=-1
        )  # [B_LOC, NH, 1024]
        arr = r["out_r"] / den[:, :, None, :]  # [B_LOC, NH, DH, HW]
        arr = arr[..., _F_OF_Q]  # undo the device-side f-ordering of hw columns
        for b in range(B_LOC):
            # flat order of reference output = q*512 + n*128 + d
            out[i * B_LOC + b] = (
                arr[b].transpose(2, 0, 1).reshape(512, 32, 32)
            )
    return out


def run(x, w_qkv, height_rel, width_rel, **spmd_kwargs):
    nc = _build()
    maps = _in_maps(x, w_qkv, height_rel, width_rel)
    res = run_bass_kernel_spmd(nc, maps, core_ids=list(range(N_CORES)), **spmd_kwargs)
    return _assemble(res.results), res


def kernel(x, w_qkv, height_rel, width_rel):
    out, _ = run(x, w_qkv, height_rel, width_rel)
    return out
